# revision 1
# baseline (speedup 1.0000x reference)
"""TRN2 Bass kernel for nn_ComboFwdVecComp (B=4, S=512, C=V=128).

out[b,i,j,v] = tanh( sum_c ctx[b,i,c]*ctx[b,j,c]*Wm[v,c]        (M term)
                     + ctx[b,j,:] @ (W1+Wd).T                    (A term, j-dep)
                     + ctx[b,i,:] @ (W2-Wd).T + (b1+b2+bm+bd)    (Brow, i-dep) )

Output (4,512,512,128) f32 = 512 MiB -> memory-bound (HBM write dominated).

Sharding: 8 cores, core k handles b = k//2, i in [ (k%2)*256, +256 ).
Each core emits out_shard (256, 512, 128) = 64 MiB; host concatenates.

Per-core structure: i is processed in "quads" (4 consecutive i), 8 quads per
qblock. PSUM is one [128, 4096] megatile (8 banks); bank = (half, s).
For each (qblock, jc): two halves of 4 quads each:
  bias mm  (K=1, N=512): ones^T @ Brow_quad -> bank, strip-tiled so the four
           bias mms run CONCURRENTLY on PE row-strips 0/32/64/96
  main mm  (K=128, N=512): ctxT_chunk_jc^T @ rhs'_quad accumulates on top.
           rhs'[c,(i,v)] = WmT[c,v]*ctxi[c,i] + AW[c,v] is prepped on DVE two
           quads at a time with two [C,1024] tensor_tensor ops (step-0
           broadcast APs carry WmT/AW across i and ctxi scalars across v)
  ACT tanh drains the half [128,2048] -> SBUF; 4 DMAs (256 KiB each) store it,
  rotating across the SP-HWDGE / ACT-HWDGE / Pool-SWDGE queues for bandwidth.

All matmuls run in float32r (TF32-like, ~1.5e-4 rel err, ~1 cyc/row at
N=512; plain fp32 is 4 cyc/row and lowers to serial LOW+HIGH pairs). fp32r
operands must come from a rounding compute op, so ctxT/ones/brow are rounded
by DVE copies and rhs' by its producing DVE add.
brow rows live on partitions {0,32,64,96}: quad q -> partition (q%4)*32,
column block q//4 (needed both for K=1 matmul base rules and strip tiling).

Measured on 8 axon trn2 cores: HW exec ~282 us = ~20 us ramp + ~253 us
HBM-write floor (64 MiB shard at the measured ~255 GB/s per-core write
ceiling). Input DMAs are ordered brow-first on both HWDGE queues because the
Tile scheduler bakes its modeled DMA completion order into semaphore waits:
with brow queued behind other inputs, the first bias matmul was gated ~40 us
on a baked DVE-stream position instead of ~13 us.
"""

import sys
import types
from contextlib import ExitStack

import numpy as np

import concourse.bass as bass
import concourse.mybir as mybir
import concourse.tile as tile
from concourse import bacc
from concourse.bass_utils import run_bass_kernel_spmd

B, S, C, V = 4, 512, 128, 128
NCORES = 8
NI = 256          # i's per core
NQ = NI // 4      # quads per core (64)
NQB = 8           # qblocks (8 quads each)

_F32 = mybir.dt.float32
_F32R = mybir.dt.float32r


def install_ntff_shim():
    """antenv.axon_hooks is absent on some images; shim it so trace=True works."""
    if "antenv.axon_hooks" in sys.modules:
        return
    try:
        from trn_agent_boot.trn_boot import _ntff_profile_via_ctypes
        hook = _ntff_profile_via_ctypes("/opt/axon/libaxon_pjrt.so")
    except Exception:
        hook = None
    mod = types.ModuleType("antenv.axon_hooks")
    mod.get_axon_ntff_profile_hook = lambda: hook
    mod.set_axon_ntff_profile_hook = lambda h: None
    sys.modules["antenv.axon_hooks"] = mod


def build_nc():
    nc = bacc.Bacc("TRN2", target_bir_lowering=False, debug=False)

    ctxT_d = nc.dram_tensor("ctxT", [C, S], _F32, kind="ExternalInput").ap()
    ctxi_d = nc.dram_tensor("ctxi", [C, NI], _F32, kind="ExternalInput").ap()
    wmT_d = nc.dram_tensor("wmT", [C, V], _F32, kind="ExternalInput").ap()
    aw_d = nc.dram_tensor("aw", [C, V], _F32, kind="ExternalInput").ap()
    # brow rows, dense: row r -> partition r*32, quad q -> row q%4, cols (q//4)*512
    browp_d = nc.dram_tensor("browp", [4, (NQ // 4) * 512], _F32, kind="ExternalInput").ap()
    out_d = nc.dram_tensor("out_shard", [NI, S, V], _F32, kind="ExternalOutput").ap()

    with tile.TileContext(nc) as tc, ExitStack() as ctx:
        singles = ctx.enter_context(tc.tile_pool(name="singles", bufs=1))
        rhs_pool = ctx.enter_context(tc.tile_pool(name="rhs", bufs=8))
        tmp_pool = ctx.enter_context(tc.tile_pool(name="tmp", bufs=3))
        psum_pool = ctx.enter_context(tc.tile_pool(name="psum", bufs=1, space="PSUM"))
        out_pool = ctx.enter_context(tc.tile_pool(name="outs", bufs=6))

        # ---- load constants ----
        ctxT_sb = singles.tile([C, S], _F32)
        ctxi_sb = singles.tile([C, NI], _F32)
        wmT_sb = singles.tile([C, V], _F32)
        aw_sb = singles.tile([C, V], _F32)
        browp_sb = singles.tile([97, (NQ // 4) * 512], _F32)
        # browp rows gate the first bias mms -> issue them FIRST on both
        # HWDGE queues (each queue is FIFO per issuing engine)
        for r in range(4):
            eng = nc.sync if r % 2 == 0 else nc.scalar
            eng.dma_start(out=browp_sb[32 * r:32 * r + 1, :], in_=browp_d[r:r + 1, :])
        nc.scalar.dma_start(out=ctxi_sb, in_=ctxi_d)
        nc.scalar.dma_start(out=wmT_sb, in_=wmT_d)
        nc.scalar.dma_start(out=aw_sb, in_=aw_d)
        nc.sync.dma_start(out=ctxT_sb, in_=ctxT_d)

        # ---- fp32r rounding (DVE copy = rounding producer). fp32 bias mms
        # lower to serial LOW+HIGH pairs (~2.3us/half on PE), so bias runs in
        # fp32r too: single pass, 4-strip concurrent (~0.8us/half). Rounding
        # is chunked so the first bias mm only waits on chunk 0. ----
        ones_f = singles.tile([97, 128], _F32)
        nc.vector.memset(ones_f, 1.0)
        ones_r = singles.tile([97, 128], _F32R)
        nc.vector.tensor_copy(ones_r, ones_f)
        browp_r = singles.tile([97, (NQ // 4) * 512], _F32R)
        RW = (NQ // 4) * 512
        RCH = RW // 8
        nc.vector.tensor_copy(browp_r[:, 0:RCH], browp_sb[:, 0:RCH])
        ctxT_r = singles.tile([C, S], _F32R)
        nc.vector.tensor_copy(ctxT_r, ctxT_sb)

        # broadcast APs for pair-wide (8 i's) prep: wmT/aw repeat over the
        # i dim (step 0), ctxi scalars repeat over the v dim (trailing step 0)
        wm_b8 = bass.AP(
            tensor=wmT_sb.tensor,
            offset=wmT_sb.offset,
            ap=[wmT_sb.ap[0], [0, 8], wmT_sb.ap[1]],
        )
        aw_b8 = bass.AP(
            tensor=aw_sb.tensor,
            offset=aw_sb.offset,
            ap=[aw_sb.ap[0], [0, 8], aw_sb.ap[1]],
        )

        # one 8-bank psum megatile; bank b occupies [:, b*512:(b+1)*512]
        P = psum_pool.tile([128, 4096], _F32, name="mega")

        dma_engines = [nc.sync, nc.gpsimd, nc.scalar]
        dma_i = 0

        def prep_pair(p):
            # rhs' for quads (2p, 2p+1): one mult + one add over [C, 8*V]
            tmp_p = tmp_pool.tile([C, 8 * V], _F32)
            ctxi_bc = bass.AP(
                tensor=ctxi_sb.tensor,
                offset=ctxi_sb.offset + 8 * p,
                ap=[ctxi_sb.ap[0], [1, 8], [0, V]],
            )
            nc.vector.tensor_tensor(
                out=tmp_p, in0=wm_b8, in1=ctxi_bc, op=mybir.AluOpType.mult
            )
            rhs_p = rhs_pool.tile([C, 8 * V], _F32R)
            nc.vector.tensor_tensor(
                out=rhs_p, in0=tmp_p, in1=aw_b8, op=mybir.AluOpType.add
            )
            return rhs_p

        def pair_slice(pairs, qq):
            return pairs[qq // 2][:, (qq % 2) * 4 * V:(qq % 2 + 1) * 4 * V]

        for qb in range(NQB):
            if qb == 0:
                # ramp: only the first half's quads before the first matmuls
                pairs = [prep_pair(0), prep_pair(1), None, None]
            else:
                pairs = [prep_pair(4 * qb + pp) for pp in range(4)]
            rhs_tiles = "unused"
            if qb == 1:
                for cc in range(1, 8):
                    nc.vector.tensor_copy(
                        browp_r[:, cc * RCH:(cc + 1) * RCH],
                        browp_sb[:, cc * RCH:(cc + 1) * RCH],
                    )

            for jc in range(4):
                for half in range(2):
                    # ---- bias mms: 4 quads, strip-concurrent ----
                    for s in range(4):
                        q = 8 * qb + 4 * half + s
                        strip = (q % 4) * 32
                        col = (q // 4) * 512
                        bank = 4 * half + s
                        nc.tensor.matmul(
                            P[:, bank * 512:(bank + 1) * 512],
                            lhsT=ones_r[strip:strip + 1, :],
                            rhs=browp_r[strip:strip + 1, col:col + 512],
                            start=True,
                            stop=False,
                            tile_position=(strip, 0),
                        )
                    # ---- main mms: one ctxT LDW per half ----
                    for s in range(4):
                        bank = 4 * half + s
                        nc.tensor.matmul(
                            P[:, bank * 512:(bank + 1) * 512],
                            lhsT=ctxT_r[:, jc * 128:(jc + 1) * 128],
                            rhs=pair_slice(pairs, 4 * half + s),
                            start=False,
                            stop=True,
                        )

                    if qb == 0 and jc == 0 and half == 0:
                        pairs[2] = prep_pair(2)
                        pairs[3] = prep_pair(3)

                    # ---- drain the half: tanh [128,2048] + 4 DMAs ----
                    ot = out_pool.tile([128, 2048], _F32)
                    nc.scalar.activation(
                        ot, P[:, half * 2048:(half + 1) * 2048],
                        mybir.ActivationFunctionType.Tanh,
                    )
                    for s in range(4):
                        q = 8 * qb + 4 * half + s
                        dst = bass.AP(
                            tensor=out_d.tensor,
                            offset=(4 * q) * S * V + jc * 128 * V,
                            ap=[[V, 128], [S * V, 4], [1, V]],
                        )
                        src = bass.AP(
                            tensor=ot.tensor,
                            offset=ot.offset + s * 512,
                            ap=[ot.ap[0], [128, 4], [1, V]],
                        )
                        eng = dma_engines[dma_i % 3]
                        dma_i += 1
                        eng.dma_start(out=dst, in_=src)

    nc.compile()
    return nc


_NC_CACHE = {}


def get_nc():
    if "nc" not in _NC_CACHE:
        _NC_CACHE["nc"] = build_nc()
    return _NC_CACHE["nc"]


def make_in_maps(ctx, W1, b1, W2, b2, Wm, bm, Wd, bd):
    ctx = np.asarray(ctx, np.float32)
    bias_all = (
        np.asarray(b1) + np.asarray(b2) + np.asarray(bm) + np.asarray(bd)
    ).astype(np.float32)
    wmT = np.ascontiguousarray(np.asarray(Wm, np.float32).T)                  # (C,V)
    aw = np.ascontiguousarray(
        (np.asarray(W1) + np.asarray(Wd)).T.astype(np.float32)
    )
    w2d = (np.asarray(W2) - np.asarray(Wd)).astype(np.float32)                # (V,C)

    in_maps = []
    for k in range(NCORES):
        b = k // 2
        i0c = (k % 2) * NI
        brow = (ctx[b, i0c:i0c + NI] @ w2d.T + bias_all).astype(np.float32)   # (NI,V)
        browp = np.zeros((4, (NQ // 4) * 512), np.float32)
        browq = brow.reshape(NQ, 512)                                          # quad rows
        for q in range(NQ):
            browp[q % 4, (q // 4) * 512:(q // 4) * 512 + 512] = browq[q]
        in_maps.append({
            "ctxT": np.ascontiguousarray(ctx[b].T),
            "ctxi": np.ascontiguousarray(ctx[b, i0c:i0c + NI].T),
            "wmT": wmT,
            "aw": aw,
            "browp": browp,
        })
    return in_maps


def run(in_maps, **kw):
    return run_bass_kernel_spmd(get_nc(), in_maps, core_ids=list(range(NCORES)), **kw)


def assemble(results):
    out = np.empty((B, S, S, V), np.float32)
    for k in range(NCORES):
        b = k // 2
        i0c = (k % 2) * NI
        out[b, i0c:i0c + NI] = results[k]["out_shard"]
    return out


def kernel(ctx, W1, b1, W2, b2, Wm, bm, Wd, bd):
    install_ntff_shim()
    in_maps = make_in_maps(ctx, W1, b1, W2, b2, Wm, bm, Wd, bd)
    res = run(in_maps)
    return assemble(res.results)



# revision 2
# speedup vs baseline: 1.1716x; 1.1716x over previous
"""TRN2 Bass kernel for nn_ComboFwdVecComp (B=4, S=512, C=V=128).

out[b,i,j,v] = tanh( sum_c ctx[b,i,c]*ctx[b,j,c]*Wm[v,c]        (M term)
                     + ctx[b,i,:] @ (W2-Wd).T                    (i-dep, folded in rhs)
                     + ctx[b,j,:] @ (W1+Wd).T + (b1+b2+bm+bd)    (arow, j-dep K=1 mms) )

Output (4,512,512,128) f32 = 512 MiB -> memory-bound (HBM write dominated).

Sharding: 8 cores, core k handles b = k//2, i in [ (k%2)*256, +256 ).
Each core emits out_shard (256, 512, 128) = 64 MiB; host concatenates.

Layout choice (the whole point of this version): psum/out partitions = i,
free dims = (j, v) which are CONTIGUOUS in HBM (out[i,j,v] has j stride V,
v stride 1). Each store DMA is [128 i, 16 j x 128 v] = 1 MiB with 8 KiB
contiguous per partition -> 128 descriptors of 8 KiB. The previous
partitions=j layout stored 512 B-granule descriptors and capped at
~233 GB/s effective (descriptor-rate bound, all 16 SDMA engines ~82% busy
at ~29 ns per 512 B descriptor); this layout runs at the HBM-per-NC
write ceiling (~358 GB/s) instead.

Per-core structure: j is processed in 16 jblocks of 32 j's (8 quads of 4).
PSUM is one [128, 4096] megatile (8 banks); bank = (half, s) covers one
j-quad. For each jblock: DVE preps rhs'[c,(j,v)] = WmT[c,v]*ctxj[c] +
W2dT[c,v] once ([C,2048] mult + add per 16 j's, f32r out), then BOTH
i-blocks (ib=0,1) consume it:
  bias mm  (K=1, N=512): ones^T @ arowp -> bank, strip-tiled so the four
           bias mms run CONCURRENTLY on PE row-strips 0/32/64/96
  main mm  (K=128, N=512): ctxiT_chunk_ib^T @ rhs'_quad accumulates on top.
  ACT tanh drains the half [128,2048] -> SBUF; ONE 1 MiB DMA stores it
  contiguously, rotating across SP-HWDGE / Pool-SWDGE / ACT-HWDGE queues.

All matmuls run in float32r (TF32-like, ~1.5e-4 rel err, ~1 cyc/row at
N=512; plain fp32 is 4 cyc/row and lowers to serial LOW+HIGH pairs). fp32r
operands must come from a rounding compute op, so ctxiT/ones/arowp are
rounded by DVE copies and rhs' by its producing DVE add.
arowp rows live on partitions {0,32,64,96}: j-quad q -> partition (q%4)*32,
column block q//4 (needed both for K=1 matmul base rules and strip tiling).
arowp f32 staging is done in two [97, 8192] halves through a bufs=1 pool so
the staging buffer (32 KiB/partition) is recycled; arowp_r f32r (64 KiB/
partition) stays resident. arowp rows are issued FIRST on both HWDGE queues
(the Tile scheduler bakes its modeled DMA completion order into semaphore
waits; see the partitions=j predecessor of this kernel).
"""

import sys
import types
from contextlib import ExitStack

import numpy as np

import concourse.bass as bass
import concourse.mybir as mybir
import concourse.tile as tile
from concourse import bacc
from concourse.bass_utils import run_bass_kernel_spmd

B, S, C, V = 4, 512, 128, 128
NCORES = 8
NI = 256          # i's per core (2 blocks of 128 partitions)
SQ = S // 4       # j-quads total (128)
NJB = 16          # jblocks (8 j-quads = 32 j's each)

_F32 = mybir.dt.float32
_F32R = mybir.dt.float32r


def install_ntff_shim():
    """antenv.axon_hooks is absent on some images; shim it so trace=True works."""
    if "antenv.axon_hooks" in sys.modules:
        return
    try:
        from trn_agent_boot.trn_boot import _ntff_profile_via_ctypes
        hook = _ntff_profile_via_ctypes("/opt/axon/libaxon_pjrt.so")
    except Exception:
        hook = None
    mod = types.ModuleType("antenv.axon_hooks")
    mod.get_axon_ntff_profile_hook = lambda: hook
    mod.set_axon_ntff_profile_hook = lambda h: None
    sys.modules["antenv.axon_hooks"] = mod


def build_nc():
    nc = bacc.Bacc("TRN2", target_bir_lowering=False, debug=False)

    ctxT_d = nc.dram_tensor("ctxT", [C, S], _F32, kind="ExternalInput").ap()
    ctxiT_d = nc.dram_tensor("ctxiT", [C, NI], _F32, kind="ExternalInput").ap()
    wmT_d = nc.dram_tensor("wmT", [C, V], _F32, kind="ExternalInput").ap()
    w2dT_d = nc.dram_tensor("w2dT", [C, V], _F32, kind="ExternalInput").ap()
    # arow rows, packed: quad q -> partition (q%4)*32, cols (q//4)*512
    arowp_d = nc.dram_tensor("arowp", [4, (SQ // 4) * 512], _F32, kind="ExternalInput").ap()
    out_d = nc.dram_tensor("out_shard", [NI, S, V], _F32, kind="ExternalOutput").ap()

    RW = (SQ // 4) * 512   # 16384 packed cols
    RH = RW // 2           # staged in two 8192-col halves
    RCH = 2048             # f32r cast chunk (covers 2 jblocks)

    with tile.TileContext(nc) as tc, ExitStack() as ctx:
        singles = ctx.enter_context(tc.tile_pool(name="singles", bufs=1))
        stage_pool = ctx.enter_context(tc.tile_pool(name="stage", bufs=1))
        rhs_pool = ctx.enter_context(tc.tile_pool(name="rhs", bufs=4))
        tmp_pool = ctx.enter_context(tc.tile_pool(name="tmp", bufs=2))
        psum_pool = ctx.enter_context(tc.tile_pool(name="psum", bufs=1, space="PSUM"))
        out_pool = ctx.enter_context(tc.tile_pool(name="outs", bufs=5))

        # ---- load constants; arowp rows FIRST on both HWDGE queues ----
        arowp_r = singles.tile([97, RW], _F32R)
        stg_a = stage_pool.tile([97, RH], _F32, name="stg")
        for r in range(4):
            eng = nc.sync if r % 2 == 0 else nc.scalar
            eng.dma_start(out=stg_a[32 * r:32 * r + 1, :], in_=arowp_d[r:r + 1, 0:RH])
        ctxiT_sb = singles.tile([C, NI], _F32)
        ctxT_sb = singles.tile([C, S], _F32)
        wmT_sb = singles.tile([C, V], _F32)
        w2dT_sb = singles.tile([C, V], _F32)
        nc.scalar.dma_start(out=ctxiT_sb, in_=ctxiT_d)
        nc.scalar.dma_start(out=wmT_sb, in_=wmT_d)
        nc.scalar.dma_start(out=w2dT_sb, in_=w2dT_d)
        nc.sync.dma_start(out=ctxT_sb, in_=ctxT_d)

        # ---- fp32r rounding, ordered so the first bias/main mms unblock
        # earliest: ones -> arowp chunk 0 -> ctxiT -> (preps) ----
        ones_f = singles.tile([97, 128], _F32)
        nc.vector.memset(ones_f, 1.0)
        ones_r = singles.tile([97, 128], _F32R)
        nc.vector.tensor_copy(ones_r, ones_f)
        nc.vector.tensor_copy(arowp_r[:, 0:RCH], stg_a[:, 0:RCH])
        ctxiT_r = singles.tile([C, NI], _F32R)
        nc.vector.tensor_copy(ctxiT_r, ctxiT_sb)

        # broadcast APs for half-jblock (16 j's) prep: wmT/w2dT repeat over
        # the j dim (step 0), ctxT j scalars repeat over the v dim (step 0)
        wm_b16 = bass.AP(
            tensor=wmT_sb.tensor,
            offset=wmT_sb.offset,
            ap=[wmT_sb.ap[0], [0, 16], wmT_sb.ap[1]],
        )
        w2d_b16 = bass.AP(
            tensor=w2dT_sb.tensor,
            offset=w2dT_sb.offset,
            ap=[w2dT_sb.ap[0], [0, 16], w2dT_sb.ap[1]],
        )

        # one 8-bank psum megatile; bank b occupies [:, b*512:(b+1)*512]
        P = psum_pool.tile([128, 4096], _F32, name="mega")

        dma_engines = [nc.sync, nc.gpsimd, nc.scalar]
        dma_i = 0

        def prep_half(jb, h):
            # rhs' for 16 j's (quads 8jb+4h .. +3): one mult + one add [C, 2048]
            j0 = 32 * jb + 16 * h
            tmp_p = tmp_pool.tile([C, 16 * V], _F32, name="tmp")
            ctxj_bc = bass.AP(
                tensor=ctxT_sb.tensor,
                offset=ctxT_sb.offset + j0,
                ap=[ctxT_sb.ap[0], [1, 16], [0, V]],
            )
            nc.vector.tensor_tensor(
                out=tmp_p, in0=wm_b16, in1=ctxj_bc, op=mybir.AluOpType.mult
            )
            rhs_p = rhs_pool.tile([C, 16 * V], _F32R, name="rhs")
            nc.vector.tensor_tensor(
                out=rhs_p, in0=tmp_p, in1=w2d_b16, op=mybir.AluOpType.add
            )
            return rhs_p

        for jb in range(NJB):
            halves = [prep_half(jb, 0), prep_half(jb, 1)]
            if jb == 0:
                # remaining f32r chunks of staging half A, then recycle the
                # staging buffer for half B (bufs=1 pool -> WAR-sequenced)
                for cc in range(1, RH // RCH):
                    nc.vector.tensor_copy(
                        arowp_r[:, cc * RCH:(cc + 1) * RCH],
                        stg_a[:, cc * RCH:(cc + 1) * RCH],
                    )
                stg_b = stage_pool.tile([97, RH], _F32, name="stg")
                for r in range(4):
                    eng = nc.sync if r % 2 == 0 else nc.scalar
                    eng.dma_start(
                        out=stg_b[32 * r:32 * r + 1, :], in_=arowp_d[r:r + 1, RH:RW]
                    )
            if jb == 1:
                for cc in range(RH // RCH):
                    nc.vector.tensor_copy(
                        arowp_r[:, RH + cc * RCH:RH + (cc + 1) * RCH],
                        stg_b[:, cc * RCH:(cc + 1) * RCH],
                    )

            for ib in range(2):
                for half in range(2):
                    # ---- bias mms: 4 j-quads, strip-concurrent ----
                    for s in range(4):
                        strip = s * 32
                        col = (2 * jb + half) * 512
                        bank = 4 * half + s
                        nc.tensor.matmul(
                            P[:, bank * 512:(bank + 1) * 512],
                            lhsT=ones_r[strip:strip + 1, :],
                            rhs=arowp_r[strip:strip + 1, col:col + 512],
                            start=True,
                            stop=False,
                            tile_position=(strip, 0),
                        )
                    # ---- main mms: one ctxiT LDW per half ----
                    for s in range(4):
                        bank = 4 * half + s
                        nc.tensor.matmul(
                            P[:, bank * 512:(bank + 1) * 512],
                            lhsT=ctxiT_r[:, ib * 128:(ib + 1) * 128],
                            rhs=halves[half][:, s * 512:(s + 1) * 512],
                            start=False,
                            stop=True,
                        )

                    # ---- drain the half: tanh [128,2048] + ONE 1 MiB DMA,
                    # contiguous 8 KiB per partition ----
                    ot = out_pool.tile([128, 2048], _F32, name="ot")
                    nc.scalar.activation(
                        ot, P[:, half * 2048:(half + 1) * 2048],
                        mybir.ActivationFunctionType.Tanh,
                    )
                    dst = bass.AP(
                        tensor=out_d.tensor,
                        offset=(ib * 128) * S * V + (32 * jb + 16 * half) * V,
                        ap=[[S * V, 128], [1, 16 * V]],
                    )
                    eng = dma_engines[dma_i % 3]
                    dma_i += 1
                    eng.dma_start(out=dst, in_=ot)

    nc.compile()
    return nc


_NC_CACHE = {}


def get_nc():
    if "nc" not in _NC_CACHE:
        _NC_CACHE["nc"] = build_nc()
    return _NC_CACHE["nc"]


def make_in_maps(ctx, W1, b1, W2, b2, Wm, bm, Wd, bd):
    ctx = np.asarray(ctx, np.float32)
    bias_all = (
        np.asarray(b1) + np.asarray(b2) + np.asarray(bm) + np.asarray(bd)
    ).astype(np.float32)
    wmT = np.ascontiguousarray(np.asarray(Wm, np.float32).T)                  # (C,V)
    w2dT = np.ascontiguousarray(
        (np.asarray(W2) - np.asarray(Wd)).T.astype(np.float32)
    )
    w1d = (np.asarray(W1) + np.asarray(Wd)).astype(np.float32)                # (V,C)

    in_maps = []
    for k in range(NCORES):
        b = k // 2
        i0c = (k % 2) * NI
        arow = (ctx[b] @ w1d.T + bias_all).astype(np.float32)                 # (S,V)
        arowp = np.zeros((4, (SQ // 4) * 512), np.float32)
        arowq = arow.reshape(SQ, 512)                                          # quad rows
        for q in range(SQ):
            arowp[q % 4, (q // 4) * 512:(q // 4) * 512 + 512] = arowq[q]
        in_maps.append({
            "ctxT": np.ascontiguousarray(ctx[b].T),
            "ctxiT": np.ascontiguousarray(ctx[b, i0c:i0c + NI].T),
            "wmT": wmT,
            "w2dT": w2dT,
            "arowp": arowp,
        })
    return in_maps


def run(in_maps, **kw):
    return run_bass_kernel_spmd(get_nc(), in_maps, core_ids=list(range(NCORES)), **kw)


def assemble(results):
    out = np.empty((B, S, S, V), np.float32)
    for k in range(NCORES):
        b = k // 2
        i0c = (k % 2) * NI
        out[b, i0c:i0c + NI] = results[k]["out_shard"]
    return out


def kernel(ctx, W1, b1, W2, b2, Wm, bm, Wd, bd):
    install_ntff_shim()
    in_maps = make_in_maps(ctx, W1, b1, W2, b2, Wm, bm, Wd, bd)
    res = run(in_maps)
    return assemble(res.results)


# revision 5
# speedup vs baseline: 1.1825x; 1.0093x over previous
"""TRN2 Bass kernel for nn_ComboFwdVecComp (B=4, S=512, C=V=128).

out[b,i,j,v] = tanh( sum_c ctx[b,i,c]*ctx[b,j,c]*Wm[v,c]        (M term)
                     + ctx[b,i,:] @ (W2-Wd).T                    (i-dep, folded in rhs)
                     + ctx[b,j,:] @ (W1+Wd).T + (b1+b2+bm+bd)    (arow, j-dep K=1 mms) )

Output (4,512,512,128) f32 = 512 MiB -> memory-bound (HBM write dominated).

Sharding: 8 cores, core k handles b = k//2, i in [ (k%2)*256, +256 ).
Each core emits out_shard (256, 512, 128) = 64 MiB; host concatenates.

Layout choice (the whole point of this version): psum/out partitions = i,
free dims = (j, v) which are CONTIGUOUS in HBM (out[i,j,v] has j stride V,
v stride 1). Each store DMA is [128 i, 16 j x 128 v] = 1 MiB with 8 KiB
contiguous per partition -> 128 descriptors of 8 KiB. The previous
partitions=j layout stored 512 B-granule descriptors and capped at
~233 GB/s effective (descriptor-rate bound, all 16 SDMA engines ~82% busy
at ~29 ns per 512 B descriptor); this layout runs at the HBM-per-NC
write ceiling (~358 GB/s) instead.

Per-core structure: j is processed in 16 jblocks of 32 j's (8 quads of 4).
PSUM is one [128, 4096] megatile (8 banks); bank = (half, s) covers one
j-quad. For each jblock: DVE preps rhs'[c,(j,v)] = WmT[c,v]*ctxj[c] +
W2dT[c,v] once ([C,2048] mult + add per 16 j's, f32r out), then BOTH
i-blocks (ib=0,1) consume it:
  bias mm  (K=1, N=512): ones^T @ arowp -> bank, strip-tiled so the four
           bias mms run CONCURRENTLY on PE row-strips 0/32/64/96
  main mm  (K=128, N=512): ctxiT_chunk_ib^T @ rhs'_quad accumulates on top.
  ACT tanh drains the half [128,2048] -> SBUF; ONE 1 MiB DMA stores it
  contiguously, rotating across SP-HWDGE / Pool-SWDGE / ACT-HWDGE queues.

All matmuls run in float32r (TF32-like, ~1.5e-4 rel err, ~1 cyc/row at
N=512; plain fp32 is 4 cyc/row and lowers to serial LOW+HIGH pairs). fp32r
operands must come from a rounding compute op, so ctxiT/ones/arowp are
rounded by DVE copies and rhs' by its producing DVE add.
arowp rows live on partitions {0,32,64,96}: j-quad q -> partition (q%4)*32,
column block q//4 (needed both for K=1 matmul base rules and strip tiling).
arowp f32 staging is done in two [97, 8192] halves through a bufs=1 pool so
the staging buffer (32 KiB/partition) is recycled; arowp_r f32r (64 KiB/
partition) stays resident. arowp rows are issued FIRST on both HWDGE queues
(the Tile scheduler bakes its modeled DMA completion order into semaphore
waits; see the partitions=j predecessor of this kernel).
"""

import sys
import types
from contextlib import ExitStack

import numpy as np

import concourse.bass as bass
import concourse.mybir as mybir
import concourse.tile as tile
from concourse import bacc
from concourse.bass_utils import run_bass_kernel_spmd

B, S, C, V = 4, 512, 128, 128
NCORES = 8
NI = 256          # i's per core (2 blocks of 128 partitions)
SQ = S // 4       # j-quads total (128)
NJB = 16          # jblocks (8 j-quads = 32 j's each)

_F32 = mybir.dt.float32
_F32R = mybir.dt.float32r


def install_ntff_shim():
    """antenv.axon_hooks is absent on some images; shim it so trace=True works."""
    if "antenv.axon_hooks" in sys.modules:
        return
    try:
        from trn_agent_boot.trn_boot import _ntff_profile_via_ctypes
        hook = _ntff_profile_via_ctypes("/opt/axon/libaxon_pjrt.so")
    except Exception:
        hook = None
    mod = types.ModuleType("antenv.axon_hooks")
    mod.get_axon_ntff_profile_hook = lambda: hook
    mod.set_axon_ntff_profile_hook = lambda h: None
    sys.modules["antenv.axon_hooks"] = mod


def build_nc():
    nc = bacc.Bacc("TRN2", target_bir_lowering=False, debug=False)

    ctxT_d = nc.dram_tensor("ctxT", [C, S], _F32, kind="ExternalInput").ap()
    ctxiT_d = nc.dram_tensor("ctxiT", [C, NI], _F32, kind="ExternalInput").ap()
    wmT_d = nc.dram_tensor("wmT", [C, V], _F32, kind="ExternalInput").ap()
    w2dT_d = nc.dram_tensor("w2dT", [C, V], _F32, kind="ExternalInput").ap()
    # arow rows, packed: quad q -> partition (q%4)*32, cols (q//4)*512
    arowp_d = nc.dram_tensor("arowp", [4, (SQ // 4) * 512], _F32, kind="ExternalInput").ap()
    out_d = nc.dram_tensor("out_shard", [NI, S, V], _F32, kind="ExternalOutput").ap()

    RW = (SQ // 4) * 512   # 16384 packed cols
    RH = RW // 2           # staged in two 8192-col halves
    RCH = 2048             # f32r cast chunk (covers 2 jblocks)

    with tile.TileContext(nc) as tc, ExitStack() as ctx:
        singles = ctx.enter_context(tc.tile_pool(name="singles", bufs=1))
        stage_pool = ctx.enter_context(tc.tile_pool(name="stage", bufs=1))
        rhs_v_pool = ctx.enter_context(tc.tile_pool(name="rhs_v", bufs=2))
        rhs_g_pool = ctx.enter_context(tc.tile_pool(name="rhs_g", bufs=2))
        tmp_v_pool = ctx.enter_context(tc.tile_pool(name="tmp_v", bufs=2))
        tmp_g_pool = ctx.enter_context(tc.tile_pool(name="tmp_g", bufs=2))
        psum_pool = ctx.enter_context(tc.tile_pool(name="psum", bufs=1, space="PSUM"))
        out_pool = ctx.enter_context(tc.tile_pool(name="outs", bufs=5))

        # ---- load constants; arowp rows FIRST on both HWDGE queues ----
        arowp_r = singles.tile([97, RW], _F32R)
        stg_a = stage_pool.tile([97, RH], _F32, name="stg")
        for r in range(4):
            eng = nc.sync if r % 2 == 0 else nc.scalar
            eng.dma_start(out=stg_a[32 * r:32 * r + 1, :], in_=arowp_d[r:r + 1, 0:RH])
        ctxiT_sb = singles.tile([C, NI], _F32)
        ctxT_sb = singles.tile([C, S], _F32)
        wmT_sb = singles.tile([C, V], _F32)
        w2dT_sb = singles.tile([C, V], _F32)
        nc.scalar.dma_start(out=ctxiT_sb, in_=ctxiT_d)
        nc.scalar.dma_start(out=wmT_sb, in_=wmT_d)
        nc.scalar.dma_start(out=w2dT_sb, in_=w2dT_d)
        nc.sync.dma_start(out=ctxT_sb, in_=ctxT_d)

        # ---- fp32r rounding, ordered so the first bias/main mms unblock
        # earliest: ones -> arowp chunk 0 -> ctxiT -> (preps) ----
        ones_f = singles.tile([97, 128], _F32)
        nc.vector.memset(ones_f, 1.0)
        ones_r = singles.tile([97, 128], _F32R)
        nc.vector.tensor_copy(ones_r, ones_f)
        nc.vector.tensor_copy(arowp_r[:, 0:RCH], stg_a[:, 0:RCH])
        ctxiT_r = singles.tile([C, NI], _F32R)
        nc.vector.tensor_copy(ctxiT_r, ctxiT_sb)

        # broadcast APs for half-jblock (16 j's) prep: wmT/w2dT repeat over
        # the j dim (step 0), ctxT j scalars repeat over the v dim (step 0)
        wm_b16 = bass.AP(
            tensor=wmT_sb.tensor,
            offset=wmT_sb.offset,
            ap=[wmT_sb.ap[0], [0, 16], wmT_sb.ap[1]],
        )
        w2d_b16 = bass.AP(
            tensor=w2dT_sb.tensor,
            offset=w2dT_sb.offset,
            ap=[w2dT_sb.ap[0], [0, 16], w2dT_sb.ap[1]],
        )

        # one 8-bank psum megatile; bank b occupies [:, b*512:(b+1)*512]
        P = psum_pool.tile([128, 4096], _F32, name="mega")

        dma_engines = [nc.sync, nc.scalar]
        dma_i = 0

        def prep_half(jb, h):
            # rhs' for 16 j's (quads 8jb+4h .. +3): one mult + one add [C, 2048].
            # h=0 runs on DVE, h=1 on GpSimd (Pool) so neither engine gates PE.
            j0 = 32 * jb + 16 * h
            eng = nc.vector if h == 0 else nc.gpsimd
            tmp_pool = tmp_v_pool if h == 0 else tmp_g_pool
            rhs_pool = rhs_v_pool if h == 0 else rhs_g_pool
            tmp_p = tmp_pool.tile([C, 16 * V], _F32, name="tmp")
            ctxj_bc = bass.AP(
                tensor=ctxT_sb.tensor,
                offset=ctxT_sb.offset + j0,
                ap=[ctxT_sb.ap[0], [1, 16], [0, V]],
            )
            eng.tensor_tensor(
                out=tmp_p, in0=wm_b16, in1=ctxj_bc, op=mybir.AluOpType.mult
            )
            rhs_p = rhs_pool.tile([C, 16 * V], _F32R, name="rhs")
            eng.tensor_tensor(
                out=rhs_p, in0=tmp_p, in1=w2d_b16, op=mybir.AluOpType.add
            )
            return rhs_p

        for jb in range(NJB):
            halves = [prep_half(jb, 0), prep_half(jb, 1)]
            if jb == 0:
                # remaining f32r chunks of staging half A, then recycle the
                # staging buffer for half B (bufs=1 pool -> WAR-sequenced)
                for cc in range(1, RH // RCH):
                    nc.vector.tensor_copy(
                        arowp_r[:, cc * RCH:(cc + 1) * RCH],
                        stg_a[:, cc * RCH:(cc + 1) * RCH],
                    )
                stg_b = stage_pool.tile([97, RH], _F32, name="stg")
                for r in range(4):
                    eng = nc.sync if r % 2 == 0 else nc.scalar
                    eng.dma_start(
                        out=stg_b[32 * r:32 * r + 1, :], in_=arowp_d[r:r + 1, RH:RW]
                    )
            if jb == 1:
                for cc in range(RH // RCH):
                    nc.vector.tensor_copy(
                        arowp_r[:, RH + cc * RCH:RH + (cc + 1) * RCH],
                        stg_b[:, cc * RCH:(cc + 1) * RCH],
                    )

            for ib in range(2):
                # ---- all 8 bias mms first (two strip-concurrent groups),
                # then all 8 main mms with a single ctxiT LDW: keeps PE in
                # one long burst per round (fewer HAM re-throttles) and cuts
                # LDWEIGHTS traffic 16x -> 5x per round ----
                for half in range(2):
                    for s in range(4):
                        strip = s * 32
                        col = (2 * jb + half) * 512
                        bank = 4 * half + s
                        nc.tensor.matmul(
                            P[:, bank * 512:(bank + 1) * 512],
                            lhsT=ones_r[strip:strip + 1, :],
                            rhs=arowp_r[strip:strip + 1, col:col + 512],
                            start=True,
                            stop=False,
                            tile_position=(strip, 0),
                        )
                for half in range(2):
                    for s in range(4):
                        bank = 4 * half + s
                        nc.tensor.matmul(
                            P[:, bank * 512:(bank + 1) * 512],
                            lhsT=ctxiT_r[:, ib * 128:(ib + 1) * 128],
                            rhs=halves[half][:, s * 512:(s + 1) * 512],
                            start=False,
                            stop=True,
                        )
                    # ---- drain the half as soon as its 4 mains are done:
                    # tanh [128,2048] + ONE 1 MiB DMA, 8 KiB/partition
                    # contiguous ----
                    ot = out_pool.tile([128, 2048], _F32, name="ot")
                    nc.scalar.activation(
                        ot, P[:, half * 2048:(half + 1) * 2048],
                        mybir.ActivationFunctionType.Tanh,
                    )
                    dst = bass.AP(
                        tensor=out_d.tensor,
                        offset=(ib * 128) * S * V + (32 * jb + 16 * half) * V,
                        ap=[[S * V, 128], [1, 16 * V]],
                    )
                    eng = dma_engines[dma_i % 2]
                    dma_i += 1
                    eng.dma_start(out=dst, in_=ot)

    nc.compile()
    return nc


_NC_CACHE = {}


def get_nc():
    if "nc" not in _NC_CACHE:
        _NC_CACHE["nc"] = build_nc()
    return _NC_CACHE["nc"]


def make_in_maps(ctx, W1, b1, W2, b2, Wm, bm, Wd, bd):
    ctx = np.asarray(ctx, np.float32)
    bias_all = (
        np.asarray(b1) + np.asarray(b2) + np.asarray(bm) + np.asarray(bd)
    ).astype(np.float32)
    wmT = np.ascontiguousarray(np.asarray(Wm, np.float32).T)                  # (C,V)
    w2dT = np.ascontiguousarray(
        (np.asarray(W2) - np.asarray(Wd)).T.astype(np.float32)
    )
    w1d = (np.asarray(W1) + np.asarray(Wd)).astype(np.float32)                # (V,C)

    in_maps = []
    for k in range(NCORES):
        b = k // 2
        i0c = (k % 2) * NI
        arow = (ctx[b] @ w1d.T + bias_all).astype(np.float32)                 # (S,V)
        arowp = np.zeros((4, (SQ // 4) * 512), np.float32)
        arowq = arow.reshape(SQ, 512)                                          # quad rows
        for q in range(SQ):
            arowp[q % 4, (q // 4) * 512:(q // 4) * 512 + 512] = arowq[q]
        in_maps.append({
            "ctxT": np.ascontiguousarray(ctx[b].T),
            "ctxiT": np.ascontiguousarray(ctx[b, i0c:i0c + NI].T),
            "wmT": wmT,
            "w2dT": w2dT,
            "arowp": arowp,
        })
    return in_maps


def run(in_maps, **kw):
    return run_bass_kernel_spmd(get_nc(), in_maps, core_ids=list(range(NCORES)), **kw)


def assemble(results):
    out = np.empty((B, S, S, V), np.float32)
    for k in range(NCORES):
        b = k // 2
        i0c = (k % 2) * NI
        out[b, i0c:i0c + NI] = results[k]["out_shard"]
    return out


def kernel(ctx, W1, b1, W2, b2, Wm, bm, Wd, bd):
    install_ntff_shim()
    in_maps = make_in_maps(ctx, W1, b1, W2, b2, Wm, bm, Wd, bd)
    res = run(in_maps)
    return assemble(res.results)


# revision 6
# speedup vs baseline: 1.3020x; 1.1011x over previous
"""TRN2 Bass kernel for nn_ComboFwdVecComp (B=4, S=512, C=V=128).

out[b,i,j,v] = tanh( sum_c ctx[b,i,c]*ctx[b,j,c]*Wm[v,c]        (M term)
                     + ctx[b,i,:] @ (W2-Wd).T                    (i-dep, folded in rhs)
                     + ctx[b,j,:] @ (W1+Wd).T + (b1+b2+bm+bd)    (arow, j-dep K=1 mms) )

Output (4,512,512,128) f32 = 512 MiB -> memory-bound (HBM write dominated).

Sharding: 8 cores, core k handles b = k//2, j in [ (k%2)*256, +256 ), ALL i.
Each core emits out_shard (512, 256, 128) = 64 MiB; host concatenates on j.

Layout: psum/out partitions = i, free dims = (j, v) which are CONTIGUOUS in
HBM. Each store DMA is [128 i, 16 j x 128 v] = 1 MiB with 8 KiB contiguous
per partition (128 descriptors of 8 KiB). A partitions=j layout stores 512 B
granules and caps at ~233 GB/s (descriptor-rate bound); this one runs at the
SDMA line rate (~420 GB/s when fed).

Why shard j (not i) across the core pair: the DVE rhs-prep
rhs'[c,(j,v)] = WmT[c,v]*ctxj[c] + W2dT[c,v] depends only on j, and is
reused by every i-block. With 256 j's and 4 i-blocks of 128 per core, each
prep is consumed 4x, so DVE does ~91 us of prep per core -- safely under
the ~165 us DMA floor. (An i-sharded core pair preps all 512 j's for only
2 i-blocks = 2x the DVE work, and DVE was becoming the bottleneck.)
Prep must stay on ONE elementwise engine: DVE 2-source ops and ANY GpSimd
op arbitrate an exclusive SBUF shared-port lock and fully serialize.

Per-core structure: j is processed in 8 jblocks of 32 j's (8 quads of 4).
PSUM is one [128, 4096] megatile (8 banks); bank = (half, s) = one j-quad.
Per jblock: DVE preps rhs' once (two [C,2048] mult+add pairs, f32r out),
then ALL FOUR i-blocks consume it:
  8 bias mms (K=1, N=512) first: ones^T @ arowp -> bank, strip-tiled on PE
    row-strips 0/32/64/96 (4 run concurrently), two groups;
  8 main mms (K=128, N=512) after, ONE ctxT LDW for all 8: ctxT_chunk_ib^T
    @ rhs'_quad accumulates on the bias. Long PE bursts + few LDWs keep the
    PE HAM clock warm.
  ACT tanh drains each half [128,2048] -> SBUF as soon as its 4 mains are
  done; ONE 1 MiB DMA per half stores it, alternating SP/ACT HWDGE queues
  (gpsimd SWDGE would get lock-blocked by DVE preps).

All matmuls run in float32r (TF32-like, ~1.5e-4 rel err, ~1 cyc/row at
N=512; plain fp32 is 4 cyc/row). fp32r operands must come from a rounding
compute op, so ctxT/ones/arowp are rounded by DVE copies and rhs' by its
producing DVE add.
arowp rows live on partitions {0,32,64,96}: j-quad q -> partition (q%4)*32,
column block q//4 (K=1 matmul base rules + strip tiling). arowp rows are
issued FIRST on both HWDGE queues (the Tile scheduler bakes its modeled DMA
completion order into semaphore waits).
"""

import sys
import types
from contextlib import ExitStack

import numpy as np

import concourse.bass as bass
import concourse.mybir as mybir
import concourse.tile as tile
from concourse import bacc
from concourse.bass_utils import run_bass_kernel_spmd

B, S, C, V = 4, 512, 128, 128
NCORES = 8
NJ = 256          # j's per core
JQ = NJ // 4      # j-quads per core (64)
NJB = 8           # jblocks (8 j-quads = 32 j's each)
NIB = 4           # i-blocks of 128 partitions (all of S)

_F32 = mybir.dt.float32
_F32R = mybir.dt.float32r


def install_ntff_shim():
    """antenv.axon_hooks is absent on some images; shim it so trace=True works."""
    if "antenv.axon_hooks" in sys.modules:
        return
    try:
        from trn_agent_boot.trn_boot import _ntff_profile_via_ctypes
        hook = _ntff_profile_via_ctypes("/opt/axon/libaxon_pjrt.so")
    except Exception:
        hook = None
    mod = types.ModuleType("antenv.axon_hooks")
    mod.get_axon_ntff_profile_hook = lambda: hook
    mod.set_axon_ntff_profile_hook = lambda h: None
    sys.modules["antenv.axon_hooks"] = mod


def build_nc():
    nc = bacc.Bacc("TRN2", target_bir_lowering=False, debug=False)

    ctxT_d = nc.dram_tensor("ctxT", [C, S], _F32, kind="ExternalInput").ap()
    ctxjT_d = nc.dram_tensor("ctxjT", [C, NJ], _F32, kind="ExternalInput").ap()
    wmT_d = nc.dram_tensor("wmT", [C, V], _F32, kind="ExternalInput").ap()
    w2dT_d = nc.dram_tensor("w2dT", [C, V], _F32, kind="ExternalInput").ap()
    # arow rows, packed: quad q -> partition (q%4)*32, cols (q//4)*512
    arowp_d = nc.dram_tensor("arowp", [4, (JQ // 4) * 512], _F32, kind="ExternalInput").ap()
    out_d = nc.dram_tensor("out_shard", [S, NJ, V], _F32, kind="ExternalOutput").ap()

    RW = (JQ // 4) * 512   # 8192 packed cols
    RCH = 2048             # f32r cast chunk (covers 2 jblocks)

    with tile.TileContext(nc) as tc, ExitStack() as ctx:
        singles = ctx.enter_context(tc.tile_pool(name="singles", bufs=1))
        rhs_pool = ctx.enter_context(tc.tile_pool(name="rhs", bufs=4))
        tmp_pool = ctx.enter_context(tc.tile_pool(name="tmp", bufs=2))
        psum_pool = ctx.enter_context(tc.tile_pool(name="psum", bufs=1, space="PSUM"))
        out_pool = ctx.enter_context(tc.tile_pool(name="outs", bufs=6))

        # ---- load constants; arowp rows FIRST on both HWDGE queues ----
        arowp_sb = singles.tile([97, RW], _F32)
        for r in range(4):
            eng = nc.sync if r % 2 == 0 else nc.scalar
            eng.dma_start(out=arowp_sb[32 * r:32 * r + 1, :], in_=arowp_d[r:r + 1, :])
        ctxjT_sb = singles.tile([C, NJ], _F32)
        ctxT_sb = singles.tile([C, S], _F32)
        wmT_sb = singles.tile([C, V], _F32)
        w2dT_sb = singles.tile([C, V], _F32)
        nc.scalar.dma_start(out=ctxjT_sb, in_=ctxjT_d)
        nc.scalar.dma_start(out=wmT_sb, in_=wmT_d)
        nc.scalar.dma_start(out=w2dT_sb, in_=w2dT_d)
        nc.sync.dma_start(out=ctxT_sb, in_=ctxT_d)

        # ---- fp32r rounding, ordered so the first bias/main mms unblock
        # earliest: ones -> arowp chunk 0 -> ctxT -> (preps) -> rest ----
        ones_f = singles.tile([97, 128], _F32)
        nc.vector.memset(ones_f, 1.0)
        ones_r = singles.tile([97, 128], _F32R)
        nc.vector.tensor_copy(ones_r, ones_f)
        arowp_r = singles.tile([97, RW], _F32R)
        nc.vector.tensor_copy(arowp_r[:, 0:RCH], arowp_sb[:, 0:RCH])
        ctxT_r = singles.tile([C, S], _F32R)
        nc.vector.tensor_copy(ctxT_r, ctxT_sb)

        # broadcast APs for half-jblock (16 j's) prep: wmT/w2dT repeat over
        # the j dim (step 0), ctxjT j scalars repeat over the v dim (step 0)
        wm_b16 = bass.AP(
            tensor=wmT_sb.tensor,
            offset=wmT_sb.offset,
            ap=[wmT_sb.ap[0], [0, 16], wmT_sb.ap[1]],
        )
        w2d_b16 = bass.AP(
            tensor=w2dT_sb.tensor,
            offset=w2dT_sb.offset,
            ap=[w2dT_sb.ap[0], [0, 16], w2dT_sb.ap[1]],
        )

        # one 8-bank psum megatile; bank b occupies [:, b*512:(b+1)*512]
        P = psum_pool.tile([128, 4096], _F32, name="mega")

        dma_engines = [nc.sync, nc.scalar]
        dma_i = 0

        def prep_half(jb, h):
            # rhs' for 16 j's (quads 8jb+4h .. +3): one mult + one add [C, 2048]
            j0 = 32 * jb + 16 * h
            tmp_p = tmp_pool.tile([C, 16 * V], _F32, name="tmp")
            ctxj_bc = bass.AP(
                tensor=ctxjT_sb.tensor,
                offset=ctxjT_sb.offset + j0,
                ap=[ctxjT_sb.ap[0], [1, 16], [0, V]],
            )
            nc.vector.tensor_tensor(
                out=tmp_p, in0=wm_b16, in1=ctxj_bc, op=mybir.AluOpType.mult
            )
            rhs_p = rhs_pool.tile([C, 16 * V], _F32R, name="rhs")
            nc.vector.tensor_tensor(
                out=rhs_p, in0=tmp_p, in1=w2d_b16, op=mybir.AluOpType.add
            )
            return rhs_p

        for jb in range(NJB):
            halves = [prep_half(jb, 0), prep_half(jb, 1)]
            if jb == 1:
                for cc in range(1, RW // RCH):
                    nc.vector.tensor_copy(
                        arowp_r[:, cc * RCH:(cc + 1) * RCH],
                        arowp_sb[:, cc * RCH:(cc + 1) * RCH],
                    )

            for ib in range(NIB):
                # ---- all 8 bias mms first (two strip-concurrent groups),
                # then all 8 main mms with a single ctxT LDW: long PE bursts
                # (fewer HAM re-throttles), 16x -> 5x fewer LDWEIGHTS ----
                for half in range(2):
                    for s in range(4):
                        strip = s * 32
                        col = (2 * jb + half) * 512
                        bank = 4 * half + s
                        nc.tensor.matmul(
                            P[:, bank * 512:(bank + 1) * 512],
                            lhsT=ones_r[strip:strip + 1, :],
                            rhs=arowp_r[strip:strip + 1, col:col + 512],
                            start=True,
                            stop=False,
                            tile_position=(strip, 0),
                        )
                for half in range(2):
                    for s in range(4):
                        bank = 4 * half + s
                        nc.tensor.matmul(
                            P[:, bank * 512:(bank + 1) * 512],
                            lhsT=ctxT_r[:, ib * 128:(ib + 1) * 128],
                            rhs=halves[half][:, s * 512:(s + 1) * 512],
                            start=False,
                            stop=True,
                        )
                    # ---- drain the half as soon as its 4 mains are done:
                    # tanh [128,2048] + ONE 1 MiB DMA, 8 KiB/partition
                    # contiguous ----
                    ot = out_pool.tile([128, 2048], _F32, name="ot")
                    nc.scalar.activation(
                        ot, P[:, half * 2048:(half + 1) * 2048],
                        mybir.ActivationFunctionType.Tanh,
                    )
                    dst = bass.AP(
                        tensor=out_d.tensor,
                        offset=(ib * 128) * NJ * V + (32 * jb + 16 * half) * V,
                        ap=[[NJ * V, 128], [1, 16 * V]],
                    )
                    eng = dma_engines[dma_i % 2]
                    dma_i += 1
                    eng.dma_start(out=dst, in_=ot)

    nc.compile()
    return nc


_NC_CACHE = {}


def get_nc():
    if "nc" not in _NC_CACHE:
        _NC_CACHE["nc"] = build_nc()
    return _NC_CACHE["nc"]


def make_in_maps(ctx, W1, b1, W2, b2, Wm, bm, Wd, bd):
    ctx = np.asarray(ctx, np.float32)
    bias_all = (
        np.asarray(b1) + np.asarray(b2) + np.asarray(bm) + np.asarray(bd)
    ).astype(np.float32)
    wmT = np.ascontiguousarray(np.asarray(Wm, np.float32).T)                  # (C,V)
    w2dT = np.ascontiguousarray(
        (np.asarray(W2) - np.asarray(Wd)).T.astype(np.float32)
    )
    w1d = (np.asarray(W1) + np.asarray(Wd)).astype(np.float32)                # (V,C)

    in_maps = []
    for k in range(NCORES):
        b = k // 2
        jo = (k % 2) * NJ
        arow = (ctx[b, jo:jo + NJ] @ w1d.T + bias_all).astype(np.float32)     # (NJ,V)
        arowp = np.zeros((4, (JQ // 4) * 512), np.float32)
        arowq = arow.reshape(JQ, 512)                                          # quad rows
        for q in range(JQ):
            arowp[q % 4, (q // 4) * 512:(q // 4) * 512 + 512] = arowq[q]
        in_maps.append({
            "ctxT": np.ascontiguousarray(ctx[b].T),
            "ctxjT": np.ascontiguousarray(ctx[b, jo:jo + NJ].T),
            "wmT": wmT,
            "w2dT": w2dT,
            "arowp": arowp,
        })
    return in_maps


def run(in_maps, **kw):
    return run_bass_kernel_spmd(get_nc(), in_maps, core_ids=list(range(NCORES)), **kw)


def assemble(results):
    out = np.empty((B, S, S, V), np.float32)
    for k in range(NCORES):
        b = k // 2
        jo = (k % 2) * NJ
        out[b, :, jo:jo + NJ] = results[k]["out_shard"]
    return out


def kernel(ctx, W1, b1, W2, b2, Wm, bm, Wd, bd):
    install_ntff_shim()
    in_maps = make_in_maps(ctx, W1, b1, W2, b2, Wm, bm, Wd, bd)
    res = run(in_maps)
    return assemble(res.results)


# revision 9
# speedup vs baseline: 1.4979x; 1.1505x over previous
"""TRN2 Bass kernel for nn_ComboFwdVecComp (B=4, S=512, C=V=128).

out[b,i,j,v] = tanh( sum_c ctx[b,i,c]*ctx[b,j,c]*Wm[v,c]        (M term)
                     + ctx[b,i,:] @ (W2-Wd).T                    (i-dep, folded in rhs)
                     + ctx[b,j,:] @ (W1+Wd).T + (b1+b2+bm+bd)    (arow, j-dep K=1 mms) )

Output (4,512,512,128) f32 = 512 MiB -> memory-bound (HBM write dominated).

Sharding: 8 cores, core k handles b = k//2, j in [ (k%2)*256, +256 ), ALL i.
Each core emits out_shard (512, 256, 128) = 64 MiB; host concatenates on j.

Layout: psum/out partitions = i, free dims = (j, v) which are CONTIGUOUS in
HBM. Each store DMA is [128 i, 16 j x 128 v] = 1 MiB with 8 KiB contiguous
per partition (128 descriptors of 8 KiB). A partitions=j layout stores 512 B
granules and caps at ~233 GB/s (descriptor-rate bound); this one runs at the
SDMA line rate (~420 GB/s when fed).

Why shard j (not i) across the core pair: the DVE rhs-prep
rhs'[c,(j,v)] = WmT[c,v]*ctxj[c] + W2dT[c,v] depends only on j, and is
reused by every i-block. With 256 j's and 4 i-blocks of 128 per core, each
prep is consumed 4x, so DVE does ~91 us of prep per core -- safely under
the ~165 us DMA floor. (An i-sharded core pair preps all 512 j's for only
2 i-blocks = 2x the DVE work, and DVE was becoming the bottleneck.)
Prep must stay on ONE elementwise engine: DVE 2-source ops and ANY GpSimd
op arbitrate an exclusive SBUF shared-port lock and fully serialize.

Per-core structure: j is processed in 8 jblocks of 32 j's (8 quads of 4).
PSUM is one [128, 4096] megatile (8 banks); bank = (half, s) = one j-quad.
Per jblock: DVE preps rhs' once (two [C,2048] mult+add pairs, f32r out),
then ALL FOUR i-blocks consume it:
  8 bias mms (K=1, N=512) first: ones^T @ arowp -> bank, strip-tiled on PE
    row-strips 0/32/64/96 (4 run concurrently), two groups;
  8 main mms (K=128, N=512) after, ONE ctxT LDW for all 8: ctxT_chunk_ib^T
    @ rhs'_quad accumulates on the bias. Long PE bursts + few LDWs keep the
    PE HAM clock warm.
  ACT tanh drains each half [128,2048] -> SBUF as soon as its 4 mains are
  done; ONE 1 MiB DMA per half stores it, alternating SP/ACT HWDGE queues
  (gpsimd SWDGE would get lock-blocked by DVE preps).

All matmuls run in float32r (TF32-like, ~1.5e-4 rel err, ~1 cyc/row at
N=512; plain fp32 is 4 cyc/row). fp32r operands must come from a rounding
compute op, so ctxT/ones/arowp are rounded by DVE copies and rhs' by its
producing DVE add.
arowp rows live on partitions {0,32,64,96}: j-quad q -> partition (q%4)*32,
column block q//4 (K=1 matmul base rules + strip tiling). arowp rows are
issued FIRST on both HWDGE queues (the Tile scheduler bakes its modeled DMA
completion order into semaphore waits).
"""

import sys
import types
from contextlib import ExitStack

import numpy as np

import concourse.bass as bass
import concourse.mybir as mybir
import concourse.tile as tile
from concourse import bacc
from concourse.bass_utils import run_bass_kernel_spmd

B, S, C, V = 4, 512, 128, 128
NCORES = 8
NJ = 256          # j's per core
JQ = NJ // 4      # j-quads per core (64)
NJB = 8           # jblocks (8 j-quads = 32 j's each)
NIB = 4           # i-blocks of 128 partitions (all of S)

_F32 = mybir.dt.float32
_F32R = mybir.dt.float32r
_BF16 = mybir.dt.bfloat16


def install_ntff_shim():
    """antenv.axon_hooks is absent on some images; shim it so trace=True works."""
    if "antenv.axon_hooks" in sys.modules:
        return
    try:
        from trn_agent_boot.trn_boot import _ntff_profile_via_ctypes
        hook = _ntff_profile_via_ctypes("/opt/axon/libaxon_pjrt.so")
    except Exception:
        hook = None
    mod = types.ModuleType("antenv.axon_hooks")
    mod.get_axon_ntff_profile_hook = lambda: hook
    mod.set_axon_ntff_profile_hook = lambda h: None
    sys.modules["antenv.axon_hooks"] = mod


def build_nc():
    nc = bacc.Bacc("TRN2", target_bir_lowering=False, debug=False)

    ctxT_d = nc.dram_tensor("ctxT", [C, S], _F32, kind="ExternalInput").ap()
    ctxjT_d = nc.dram_tensor("ctxjT", [C, NJ], _F32, kind="ExternalInput").ap()
    wmT_d = nc.dram_tensor("wmT", [C, V], _F32, kind="ExternalInput").ap()
    w2dT_d = nc.dram_tensor("w2dT", [C, V], _F32, kind="ExternalInput").ap()
    # arow rows, packed: quad q -> partition (q%4)*32, cols (q//4)*512
    arowp_d = nc.dram_tensor("arowp", [4, (JQ // 4) * 512], _F32, kind="ExternalInput").ap()
    out_d = nc.dram_tensor("out_shard", [S, NJ, V], _F32, kind="ExternalOutput").ap()

    RW = (JQ // 4) * 512   # 8192 packed cols
    RCH = 2048             # f32r cast chunk (covers 2 jblocks)

    with tile.TileContext(nc) as tc, ExitStack() as ctx:
        singles = ctx.enter_context(tc.tile_pool(name="singles", bufs=1))
        rhs_pool = ctx.enter_context(tc.tile_pool(name="rhs", bufs=4))
        tmp_pool = ctx.enter_context(tc.tile_pool(name="tmp", bufs=2))
        psum_pool = ctx.enter_context(tc.tile_pool(name="psum", bufs=1, space="PSUM"))
        out_pool = ctx.enter_context(tc.tile_pool(name="outs", bufs=6))

        # ---- load constants; arowp rows FIRST on both HWDGE queues ----
        arowp_sb = singles.tile([97, RW], _F32)
        for r in range(4):
            eng = nc.sync if r % 2 == 0 else nc.scalar
            eng.dma_start(out=arowp_sb[32 * r:32 * r + 1, :], in_=arowp_d[r:r + 1, :])
        ctxjT_sb = singles.tile([C, NJ], _F32)
        ctxT_sb = singles.tile([C, S], _F32)
        wmT_sb = singles.tile([C, V], _F32)
        w2dT_sb = singles.tile([C, V], _F32)
        nc.scalar.dma_start(out=ctxjT_sb, in_=ctxjT_d)
        nc.scalar.dma_start(out=wmT_sb, in_=wmT_d)
        nc.scalar.dma_start(out=w2dT_sb, in_=w2dT_d)
        nc.sync.dma_start(out=ctxT_sb, in_=ctxT_d)

        # ---- fp32r rounding, ordered so the first bias/main mms unblock
        # earliest: ones -> arowp chunk 0 -> ctxT -> (preps) -> rest ----
        ones_f = singles.tile([97, 128], _F32)
        nc.vector.memset(ones_f, 1.0)
        ones_r = singles.tile([97, 128], _F32R)
        nc.vector.tensor_copy(ones_r, ones_f)
        arowp_r = singles.tile([97, RW], _F32R)
        nc.vector.tensor_copy(arowp_r[:, 0:RCH], arowp_sb[:, 0:RCH])
        # main mms run in bf16: the moving operand streams 2 cols/cycle vs
        # 1 for fp32r (~215 ns vs ~427 ns per N=512 mm cold) and PE was the
        # critical path. Bias mms stay f32r so arow/biases stay exact.
        ctxT_r = singles.tile([C, S], _BF16)
        nc.vector.tensor_copy(ctxT_r, ctxT_sb)

        # broadcast APs for half-jblock (16 j's) prep: wmT/w2dT repeat over
        # the j dim (step 0), ctxjT j scalars repeat over the v dim (step 0)
        wm_b16 = bass.AP(
            tensor=wmT_sb.tensor,
            offset=wmT_sb.offset,
            ap=[wmT_sb.ap[0], [0, 16], wmT_sb.ap[1]],
        )
        w2d_b16 = bass.AP(
            tensor=w2dT_sb.tensor,
            offset=w2dT_sb.offset,
            ap=[w2dT_sb.ap[0], [0, 16], w2dT_sb.ap[1]],
        )

        # one 8-bank psum megatile; bank b occupies [:, b*512:(b+1)*512]
        P = psum_pool.tile([128, 4096], _F32, name="mega")

        dma_engines = [nc.sync, nc.scalar]
        dma_i = 0

        def prep_half(jb, h):
            # rhs' for 16 j's (quads 8jb+4h .. +3): one mult + one add [C, 2048]
            j0 = 32 * jb + 16 * h
            tmp_p = tmp_pool.tile([C, 16 * V], _F32, name="tmp")
            ctxj_bc = bass.AP(
                tensor=ctxjT_sb.tensor,
                offset=ctxjT_sb.offset + j0,
                ap=[ctxjT_sb.ap[0], [1, 16], [0, V]],
            )
            nc.vector.tensor_tensor(
                out=tmp_p, in0=wm_b16, in1=ctxj_bc, op=mybir.AluOpType.mult
            )
            rhs_p = rhs_pool.tile([C, 16 * V], _BF16, name="rhs")
            nc.vector.tensor_tensor(
                out=rhs_p, in0=tmp_p, in1=w2d_b16, op=mybir.AluOpType.add
            )
            return rhs_p

        for jb in range(NJB):
            halves = [prep_half(jb, 0), prep_half(jb, 1)]
            if jb == 1:
                for cc in range(1, RW // RCH):
                    nc.vector.tensor_copy(
                        arowp_r[:, cc * RCH:(cc + 1) * RCH],
                        arowp_sb[:, cc * RCH:(cc + 1) * RCH],
                    )

            for ib in range(NIB):
                # ---- all 8 bias mms first (two strip-concurrent groups),
                # then all 8 main mms with a single ctxT LDW: long PE bursts
                # (fewer HAM re-throttles), 16x -> 5x fewer LDWEIGHTS ----
                for half in range(2):
                    for s in range(4):
                        strip = s * 32
                        col = (2 * jb + half) * 512
                        bank = 4 * half + s
                        nc.tensor.matmul(
                            P[:, bank * 512:(bank + 1) * 512],
                            lhsT=ones_r[strip:strip + 1, :],
                            rhs=arowp_r[strip:strip + 1, col:col + 512],
                            start=True,
                            stop=False,
                            tile_position=(strip, 0),
                        )
                for half in range(2):
                    for s in range(4):
                        bank = 4 * half + s
                        nc.tensor.matmul(
                            P[:, bank * 512:(bank + 1) * 512],
                            lhsT=ctxT_r[:, ib * 128:(ib + 1) * 128],
                            rhs=halves[half][:, s * 512:(s + 1) * 512],
                            start=False,
                            stop=True,
                        )
                    # ---- drain the half as soon as its 4 mains are done:
                    # tanh [128,2048] + ONE 1 MiB DMA, 8 KiB/partition
                    # contiguous ----
                    ot = out_pool.tile([128, 2048], _F32, name="ot")
                    nc.scalar.activation(
                        ot, P[:, half * 2048:(half + 1) * 2048],
                        mybir.ActivationFunctionType.Tanh,
                    )
                    dst = bass.AP(
                        tensor=out_d.tensor,
                        offset=(ib * 128) * NJ * V + (32 * jb + 16 * half) * V,
                        ap=[[NJ * V, 128], [1, 16 * V]],
                    )
                    eng = dma_engines[dma_i % 2]
                    dma_i += 1
                    eng.dma_start(out=dst, in_=ot)

    nc.compile()
    return nc


_NC_CACHE = {}


def get_nc():
    if "nc" not in _NC_CACHE:
        _NC_CACHE["nc"] = build_nc()
    return _NC_CACHE["nc"]


def make_in_maps(ctx, W1, b1, W2, b2, Wm, bm, Wd, bd):
    ctx = np.asarray(ctx, np.float32)
    bias_all = (
        np.asarray(b1) + np.asarray(b2) + np.asarray(bm) + np.asarray(bd)
    ).astype(np.float32)
    wmT = np.ascontiguousarray(np.asarray(Wm, np.float32).T)                  # (C,V)
    w2dT = np.ascontiguousarray(
        (np.asarray(W2) - np.asarray(Wd)).T.astype(np.float32)
    )
    w1d = (np.asarray(W1) + np.asarray(Wd)).astype(np.float32)                # (V,C)

    in_maps = []
    for k in range(NCORES):
        b = k // 2
        jo = (k % 2) * NJ
        arow = (ctx[b, jo:jo + NJ] @ w1d.T + bias_all).astype(np.float32)     # (NJ,V)
        arowp = np.zeros((4, (JQ // 4) * 512), np.float32)
        arowq = arow.reshape(JQ, 512)                                          # quad rows
        for q in range(JQ):
            arowp[q % 4, (q // 4) * 512:(q // 4) * 512 + 512] = arowq[q]
        in_maps.append({
            "ctxT": np.ascontiguousarray(ctx[b].T),
            "ctxjT": np.ascontiguousarray(ctx[b, jo:jo + NJ].T),
            "wmT": wmT,
            "w2dT": w2dT,
            "arowp": arowp,
        })
    return in_maps


def run(in_maps, **kw):
    return run_bass_kernel_spmd(get_nc(), in_maps, core_ids=list(range(NCORES)), **kw)


def assemble(results):
    out = np.empty((B, S, S, V), np.float32)
    for k in range(NCORES):
        b = k // 2
        jo = (k % 2) * NJ
        out[b, :, jo:jo + NJ] = results[k]["out_shard"]
    return out


def kernel(ctx, W1, b1, W2, b2, Wm, bm, Wd, bd):
    install_ntff_shim()
    in_maps = make_in_maps(ctx, W1, b1, W2, b2, Wm, bm, Wd, bd)
    res = run(in_maps)
    return assemble(res.results)
